# revision 4
# baseline (speedup 1.0000x reference)
"""Trainium2 Bass kernel for nn_MultiHeadAttention_59227599012491.

Reference computation (per batch b):
    xf = x[b].reshape(S, 256)
    q  = softplus(xf @ Wq.T + bq);  k = softplus(xf @ Wk.T + bk)
    v  = xf @ Wv.T + bv
    out = ((q @ k.T) @ v) @ Wo.T + bo          (no softmax!)

No softmax -> attention is associative, and v is linear in x, so v and
the whole G/M weight chain fold away:
    out = q @ M + bo
    HT[c,e] = sum_s x[s,c] k[s,e]          (lhsT = x_nat chunk, rhs = k tile)
    M[e,do] = sum_c HT[c,e] WVO[c,do] + sum_s' Kbar_j[s',e] u[do]
with WVO = WvT @ WoT and u = Wo bv both host-precomputed, and
Kbar_j = per-softplus-batch partition-folded k sums (accumulated on the
otherwise-idle GPSIMD engine, 8 independent short chains so the last
one lands right after the last softplus batch; each Kbar_j is one extra
128-contract channel of the M matmul against a host-replicated u tile).

vs v1 this removes the v projection (PE -7us), the GT matmul layer, and
half the PSUM->SBUF eviction volume (DVE 22us -> ~12us); the extra
x_nat load (+2MB, natural layout) rides free DMA bandwidth. ACT
(softplus = Exp+Ln over k full + q half, 3.15M elem-passes) paces the
loop, so q-projection blocks are source-interleaved between k tiles to
keep ACT packed, and 6 dependency-free priming matmuls on a scratch
bank ramp the PE clock out of its low pstate during the DMA head.

Sharding: B=4 batches x 2 query-halves -> 8 cores, no collectives.
x_nat for h=1 cores is rotated identically to xbT: the H matmul pairs
x_nat tile t with k tile t row-for-row.

Layouts (PE computes out = lhsT.T @ rhs, contracting partition dim):
    xbT  [256, 4096]  x[b] transposed on host (queries first SQ cols)
    xnat [4096, 256]  x[b] natural row-major (same rotation as xbT)
    k    [4096, 256]  32 tiles; psum + bk on DVE, softplus on ACT
                      (batch-size ramp 2,4,8,8 then 4,2,2,2)
    qT   [256, 2048]  lhsT = Wq chunk; softplus fused w/ per-part bias
    outT [256, 2048]  lhsT = M block; bo per-partition on DVE; fp16
                      dump, host un-transposes

Scheduling facts carried over (measured): single sync-ring FIFO
position is the DMA priority mechanism; first k tile's x columns are
loaded as a dedicated small chunk so the first matmul starts ~2us
earlier; PSUM accumulation groups must not share a bank (H chunks get
a full bank each); activation tables steered so the one table holding
Exp AND Ln loads exactly once.
"""

import numpy as np

S = 4096
SQ = 2048  # query rows per core
D = 256
P = 128
IT = D // P  # 2 input-dim tiles
DT = D // P  # 2 d-model tiles
NS = S // P  # 32 sequence tiles
BLK = 512  # free-dim block for qT
N_CORES = 8
NKB = 8  # independent Kbar partial chains (one per softplus batch)

MM_DTYPE_NAME = "float16"

_CACHE = {}


def _patched_act_tables(orig_fn):
    def patched(arch):
        tabs = orig_fn(arch)
        return {
            name: (s if name == "natural_log_exp_and_others" else set())
            for name, s in tabs.items()
        }

    return patched


def _build_nc():
    import concourse.bacc as bacc
    import concourse.mybir as mybir
    import concourse.tile as tile

    FP = mybir.dt.float32
    FR = getattr(mybir.dt, MM_DTYPE_NAME)
    AF = mybir.ActivationFunctionType
    ADD = mybir.AluOpType.add

    nc = bacc.Bacc("TRN2", target_bir_lowering=False, debug=False, num_devices=1)

    xbT_d = nc.declare_dram_parameter("xbT", [D, S], FR, isOutput=False)
    xnat_d = nc.declare_dram_parameter("xnat", [S, D], FR, isOutput=False)
    # WkT it-blocks side by side: [128, (it0 256 | it1 256)]
    wkp_d = nc.declare_dram_parameter("wkp", [P, IT * D], FR, isOutput=False)
    # wq it-blocks | wo-replaced-by-WVO c-blocks: [128, 1024]
    #   cols 0:512   = WqT it-chunks (as v1)
    #   cols 512:1024 = WVO = WvT @ WoT c-chunks [c0 256do | c1 256do]
    wqop_d = nc.declare_dram_parameter("wqop", [P, 1024], FR, isOutput=False)
    # u = Wo bv replicated [128, 256]
    ut_d = nc.declare_dram_parameter("ut", [P, D], FR, isOutput=False)
    # biases: cols 0:4 = bqT|boT dt-chunks, 4:260 = bk row replicated
    bias_d = nc.declare_dram_parameter("biasc", [P, 4 + D], FP, isOutput=False)
    outp_d = nc.declare_dram_parameter("outp", [P, 2 * SQ], FR, isOutput=True)

    def mm(psum, lhsT, rhs, start, stop):
        nc.tensor.matmul(psum, lhsT, rhs, start=start, stop=stop)

    with tile.TileContext(nc) as tc:
        with (
            tc.tile_pool(name="w", bufs=1) as wpool,
            tc.tile_pool(name="big", bufs=1) as big,
            tc.tile_pool(name="tmp", bufs=4) as tpool,
            tc.tile_pool(name="psQ", bufs=2, space="PSUM") as psQ,
            tc.tile_pool(name="psK", bufs=3, space="PSUM") as psK,
            tc.tile_pool(name="psH", bufs=1, space="PSUM") as psH,
            tc.tile_pool(name="psP", bufs=1, space="PSUM") as psP,
        ):
            wk_sb = wpool.tile([P, IT * D], FR, tag="wk")
            wqo_sb = wpool.tile([P, 1024], FR, tag="wqo")
            ut_sb = wpool.tile([P, D], FR, tag="ut")
            xbT_sb = big.tile([P, IT, S], FR, tag="xbT")
            xnat_sb = big.tile([P, NS, D], FR, tag="xnat")
            biasc = wpool.tile([P, 4 + D], FP, tag="biasc")
            bias_sb = biasc[:, 0:4]
            bk_bc = biasc[:, 4 : 4 + D]
            prime_sb = wpool.tile([P, 512], FR, tag="prime")

            # --- PE pstate priming: dependency-free matmuls on a scratch
            # bank ramp the clock during the DMA head ---
            nc.gpsimd.memset(prime_sb[:, :], 0.0)
            psprime = psP.tile([P, 512], FP, tag="psP")
            for i in range(6):
                mm(psprime[:, :], prime_sb[:, 0:P], prime_sb[:, :], True, True)

            # --- input DMAs, sync-ring FIFO order is priority ---
            # first k tile's x columns as a dedicated small chunk
            for it in range(IT):
                nc.sync.dma_start(
                    xbT_sb[:, it, 0:256], xbT_d.ap()[it * P : (it + 1) * P, 0:256]
                )
            nc.sync.dma_start(wk_sb[:, :], wkp_d.ap()[:, :])
            nc.sync.dma_start(biasc[:, :], bias_d.ap()[:, :])
            for it in range(IT):
                nc.sync.dma_start(
                    xbT_sb[:, it, 256:1024], xbT_d.ap()[it * P : (it + 1) * P, 256:1024]
                )
            nc.sync.dma_start(wqo_sb[:, :], wqop_d.ap()[:, :])
            for it in range(IT):
                nc.sync.dma_start(
                    xbT_sb[:, it, 1024:2048], xbT_d.ap()[it * P : (it + 1) * P, 1024:2048]
                )
            # x natural-layout tiles: DRAM rows (128 t + p) -> [p, t, c]
            xn_ap = xnat_d.ap().rearrange("(t p) c -> p t c", p=P)
            nc.sync.dma_start(xnat_sb[:, 0:8, :], xn_ap[:, 0:8, :])
            for it in range(IT):
                nc.sync.dma_start(
                    xbT_sb[:, it, 2048:3072], xbT_d.ap()[it * P : (it + 1) * P, 2048:3072]
                )
            nc.sync.dma_start(xnat_sb[:, 8:16, :], xn_ap[:, 8:16, :])
            for it in range(IT):
                nc.sync.dma_start(
                    xbT_sb[:, it, 3072:4096], xbT_d.ap()[it * P : (it + 1) * P, 3072:4096]
                )
            nc.sync.dma_start(xnat_sb[:, 16:24, :], xn_ap[:, 16:24, :])
            nc.sync.dma_start(xnat_sb[:, 24:32, :], xn_ap[:, 24:32, :])
            nc.sync.dma_start(ut_sb[:, :], ut_d.ap()[:, :])

            k_sb = big.tile([P, NS, D], FR, tag="k")
            qT_sb = big.tile([P, DT, SQ], FR, tag="qT")
            outT_sb = big.tile([P, DT, SQ], FR, tag="outT")
            HT_sb = wpool.tile([P, IT, D], FR, tag="HT")
            M_sb = wpool.tile([P, DT, D], FR, tag="M")
            Kb_sb = wpool.tile([P, NKB, D], FR, tag="Kb")

            # persistent H accumulators: one full PSUM bank per c-chunk so the
            # two 32-tile accumulation groups never share a bank
            psH0 = psH.tile([P, 512], FP, tag="psH0")
            psH1 = psH.tile([P, 512], FP, tag="psH1")
            psHc = [psH0, psH1]

            # batch-size ramp: small first so the saturated ACT engine starts
            # ASAP, small last so the final softplus->H->M->out chain stays
            # short. batch index j doubles as the Kbar partial index.
            SPB = {1: 2, 5: 4, 13: 8, 21: 8, 25: 4, 27: 2, 29: 2, 31: 2}

            def q_block(dt, half):
                tmp = tpool.tile([P, 2, BLK], FP, tag="tmpq")
                for c in range(2):
                    blk = 2 * half + c
                    ss = slice(blk * BLK, (blk + 1) * BLK)
                    ps = psQ.tile([P, BLK], FP, tag="psQ")
                    for it in range(IT):
                        mm(
                            ps[:, :],
                            wqo_sb[:, it * D + dt * P : it * D + (dt + 1) * P],
                            xbT_sb[:, it, ss],
                            it == 0,
                            it == IT - 1,
                        )
                    nc.scalar.activation(
                        tmp[:, c, :], ps[:, :], AF.Exp, bias=bias_sb[:, dt : dt + 1]
                    )
                nc.scalar.activation(
                    qT_sb[:, dt, 2 * half * BLK : 2 * (half + 1) * BLK],
                    tmp[:, :, :].rearrange("p a b -> p (a b)"),
                    AF.Ln,
                    bias=1.0,
                )

            # q blocks interleaved between k tiles: ACT fills its gaps with
            # q softplus but the last ACT op stays the last k batch
            QAT = {3: (0, 0), 7: (0, 1), 11: (1, 0), 15: (1, 1)}

            jbatch = 0
            for t in range(NS):
                ts = slice(t * P, (t + 1) * P)
                ps = psK.tile([P, 512], FP, tag="psK")
                for it in range(IT):
                    mm(
                        ps[:, 0:D],
                        xbT_sb[:, it, ts],
                        wk_sb[:, it * D : (it + 1) * D],
                        it == 0,
                        it == IT - 1,
                    )
                nc.vector.tensor_tensor(k_sb[:, t, :], ps[:, 0:D], bk_bc, op=ADD)
                bsz = SPB.get(t, 0)
                if bsz:
                    tt = slice(t - bsz + 1, t + 1)
                    tmp = tpool.tile([P, bsz, D], FP, tag=f"tmpk{bsz}")
                    nc.scalar.activation(tmp[:, :, :], k_sb[:, tt, :], AF.Exp)
                    nc.scalar.activation(k_sb[:, tt, :], tmp[:, :, :], AF.Ln, bias=1.0)
                    for i, t2 in enumerate(range(t - bsz + 1, t + 1)):
                        for c in range(IT):
                            mm(
                                psHc[c][:, 0:D],
                                xnat_sb[:, t2, c * P : (c + 1) * P],
                                k_sb[:, t2, :],
                                t2 == 0,
                                t2 == NS - 1,
                            )
                        # per-batch Kbar partial on the idle GPSIMD engine
                        if i == 0:
                            nc.gpsimd.tensor_copy(Kb_sb[:, jbatch, :], k_sb[:, t2, :])
                        else:
                            nc.gpsimd.tensor_tensor(
                                Kb_sb[:, jbatch, :],
                                Kb_sb[:, jbatch, :],
                                k_sb[:, t2, :],
                                op=ADD,
                            )
                    jbatch += 1
                if t in QAT:
                    q_block(*QAT[t])

            # evict H, then M = HT^T WVO + sum_j Kb_j^T u
            for c in range(IT):
                nc.vector.tensor_copy(HT_sb[:, c, :], psHc[c][:, 0:D])
            for et in range(DT):
                es = slice(et * P, (et + 1) * P)
                ps = psK.tile([P, 512], FP, tag="psK")
                for c in range(IT):
                    mm(
                        ps[:, 0:D],
                        HT_sb[:, c, es],
                        wqo_sb[:, 512 + c * D : 512 + (c + 1) * D],
                        c == 0,
                        False,
                    )
                for j in range(NKB):
                    mm(
                        ps[:, 0:D],
                        Kb_sb[:, j, es],
                        ut_sb[:, :],
                        False,
                        j == NKB - 1,
                    )
                nc.vector.tensor_copy(M_sb[:, et, :], ps[:, 0:D])

            # outT[do, s] = M^T q^T + bo: lhsT = M block (stationary), bo is
            # per-partition on the DVE, fp16 transposed dump
            for dot in range(DT):
                for blk in range(SQ // BLK):
                    ss = slice(blk * BLK, (blk + 1) * BLK)
                    ps = psQ.tile([P, BLK], FP, tag="psQ")
                    for et in range(DT):
                        mm(
                            ps[:, :],
                            M_sb[:, et, dot * P : (dot + 1) * P],
                            qT_sb[:, et, ss],
                            et == 0,
                            et == DT - 1,
                        )
                    if dot == DT - 1 and blk == SQ // BLK - 1:
                        # very last chunk: split the eviction across the idle
                        # ACT and DVE so the serial tail halves
                        nc.scalar.activation(
                            outT_sb[:, dot, blk * BLK : blk * BLK + 256],
                            ps[:, 0:256],
                            AF.Identity,
                            bias=bias_sb[:, 2 + dot : 3 + dot],
                        )
                        nc.vector.tensor_scalar_add(
                            outT_sb[:, dot, blk * BLK + 256 : (blk + 1) * BLK],
                            ps[:, 256:512],
                            bias_sb[:, 2 + dot : 3 + dot],
                        )
                    else:
                        nc.vector.tensor_scalar_add(
                            outT_sb[:, dot, ss], ps[:, :], bias_sb[:, 2 + dot : 3 + dot]
                        )
                    if dot == DT - 1 and blk >= SQ // BLK - 2:
                        # ship the last two chunks individually so only one
                        # chunk's descriptors remain after the final eviction
                        off = dot * SQ + blk * BLK
                        src_ap = outT_sb[:, dot, blk * BLK : (blk + 1) * BLK]
                        if blk == SQ // BLK - 1:
                            nc.sync.dma_start(
                                outp_d.ap()[0:64, off : off + BLK], src_ap[0:64, :]
                            )
                            nc.scalar.dma_start(
                                outp_d.ap()[64:P, off : off + BLK], src_ap[64:P, :]
                            )
                        else:
                            nc.sync.dma_start(outp_d.ap()[:, off : off + BLK], src_ap)
                    elif blk % 2 == 1:
                        off = dot * SQ + (blk - 1) * BLK
                        src_ap = outT_sb[:, dot, (blk - 1) * BLK : (blk + 1) * BLK]
                        nc.sync.dma_start(outp_d.ap()[:, off : off + 2 * BLK], src_ap)

    import concourse.hw_specs as hw_specs

    orig = bacc.get_activation_tables
    bacc.get_activation_tables = _patched_act_tables(hw_specs.get_activation_tables)
    try:
        nc.compile()
    finally:
        bacc.get_activation_tables = orig
    return nc


def _get_nc():
    nc = _CACHE.get("nc")
    if nc is None:
        nc = _build_nc()
        _CACHE["nc"] = nc
    return nc


def make_in_maps(x, Wq, bq, Wk, bk, Wv, bv, Wo, bo):
    B = x.shape[0]
    mmnp = np.float16
    xf = np.asarray(x, dtype=np.float32).reshape(B, S, D)
    xfT = np.ascontiguousarray(xf.transpose(0, 2, 1).astype(mmnp))
    xnat = np.ascontiguousarray(xf.astype(mmnp))
    wk2 = np.asarray(Wk, mmnp).T
    wkp = np.ascontiguousarray(np.hstack([wk2[0:P], wk2[P:D]]))  # [128, 512]
    wq2 = np.asarray(Wq, mmnp).T
    # WVO = WvT @ WoT in fp32, then fp16
    wvo = (np.asarray(Wv, np.float64).T @ np.asarray(Wo, np.float64).T).astype(mmnp)
    wqop = np.ascontiguousarray(
        np.hstack([wq2[0:P], wq2[P:D], wvo[0:P], wvo[P:D]])
    )  # [128, (it0 wq|it1 wq|c0 WVO|c1 WVO)]
    u = (np.asarray(Wo, np.float64) @ np.asarray(bv, np.float64)).astype(mmnp)
    ut = np.ascontiguousarray(np.tile(u, (P, 1)))
    biasc = np.ascontiguousarray(
        np.hstack(
            [
                np.stack(
                    [
                        np.asarray(bq, np.float32)[0:P],
                        np.asarray(bq, np.float32)[P:D],
                        np.asarray(bo, np.float32)[0:P],
                        np.asarray(bo, np.float32)[P:D],
                    ],
                    axis=1,
                ),
                np.tile(np.asarray(bk, np.float32), (P, 1)),
            ]
        )
    )
    shared = {
        "wkp": wkp,
        "wqop": wqop,
        "ut": ut,
        "biasc": biasc,
    }
    in_maps = []
    for c in range(N_CORES):
        b, h = divmod(c, 2)
        xT = xfT[b]
        xn = xnat[b]
        if h == 1:
            # query rows first; xnat must match xbT's s-order since the H
            # matmul pairs x_nat tile t with k tile t row-for-row
            xT = np.concatenate([xT[:, SQ:], xT[:, :SQ]], axis=1)
            xn = np.concatenate([xn[SQ:], xn[:SQ]], axis=0)
        in_maps.append(
            {"xbT": np.ascontiguousarray(xT), "xnat": np.ascontiguousarray(xn), **shared}
        )
    return in_maps


def assemble_out(results, x_shape):
    B, S_, H, W = x_shape
    out = np.empty((B, S_, D), np.float32)
    for c in range(N_CORES):
        b, h = divmod(c, 2)
        outp = results[c]["outp"]  # [128, 2*SQ] fp16: [p, dot*SQ + s]
        v = outp.reshape(P, DT, SQ).astype(np.float32)
        out[b, h * SQ : (h + 1) * SQ] = v.transpose(2, 1, 0).reshape(SQ, D)
    return out.reshape(B, S_, H, W)


def kernel(x, Wq, bq, Wk, bk, Wv, bv, Wo, bo, _trace=False):
    from concourse.bass_utils import run_bass_kernel_spmd

    nc = _get_nc()
    in_maps = make_in_maps(x, Wq, bq, Wk, bk, Wv, bv, Wo, bo)
    res = run_bass_kernel_spmd(nc, in_maps, list(range(N_CORES)), trace=_trace)
    out = assemble_out(res.results, x.shape)
    if _trace:
        _CACHE["last_result"] = res
    return out
